# revision 1
# baseline (speedup 1.0000x reference)
"""BFLinear (block-floating-point quantized linear) Trainium2 kernel.

Computes: out = bf_quant(bf_quant(x) @ bf_quant(W).T + 2*b)
where bf_quant quantizes groups of 32 along the last axis to a shared
power-of-two exponent with 8 mantissa bits (values = int8 * 2^(e-7)).

Distribution over 8 NeuronCores:
  - batch dim of x sharded 8 ways (1024 rows/core)
  - W quantization split by output rows (512 rows/core); the quantized
    slab is transposed ON-CHIP (PE transposes; the PE is idle in this
    phase) into [in, out_slice] layout, stored, and AllGathered once.
    Keeping this chain on-chip lets the AllGather trigger early.
  - x is quantized and PE-transposed on-chip into a resident SBUF xqT
    (no DRAM round-trip; the XBAR DMA-transpose path measured ~30GB/s
    and starved the matmul-phase weight feed)
  - queues: qAct(scalar) carries the W chain + output stores; qSp(sync)
    carries x loads + matmul-phase weight loads (contiguous 128KB)
  - matmul runs in bf16 (exact products, fp32 PSUM accumulation; the
    dot products are exactly representable in fp32 for this data)
  - the AllGather window is covered by the x quantization; the matmul
    phase runs two half-batch passes of 4 chains with 2-deep PSUM
    double-buffering, PSUM banks released via scalar-engine copies

Quantization math (all exact, matching jnp semantics, verified on HW):
  m     = max |x| over each group of 32          (abs-max reduce)
  scale = 2^(floor(log2 m) - 7)                  (exponent-field bit math)
  inv   = 1/scale                                (bit math, exact)
  r     = rne_round(clamp(x*inv)) via +C trick with C = 1.5*2^23
  q     = (r - C) * scale
"""

import numpy as np

# full-problem dimensions (hardcoded per harness contract)
B_FULL = 8192
IN_FULL = 4096
OUT_FULL = 4096
NCORES = 8

P = 128
SZ = 32
NB = 512  # output column block (= w_sl for 8 cores)
C_RND = float(3 * 2**22)  # 1.5*2^23: v+C stays in [2^23, 2^24) -> RNE to ints


def build_nc(b_sh=B_FULL // NCORES, in_dim=IN_FULL, out_dim=OUT_FULL,
             ncores=NCORES, for_timeline=False):
    """Build the SPMD Bass program (identical on every core; data differs)."""
    import concourse.mybir as mybir
    import concourse.tile as tile
    from concourse import bacc

    F32 = mybir.dt.float32
    BF16 = mybir.dt.bfloat16
    I32 = mybir.dt.int32
    ALU = mybir.AluOpType
    AX = mybir.AxisListType
    AF = mybir.ActivationFunctionType

    w_sl = out_dim // ncores          # W rows quantized on this core
    kc = in_dim // P                  # 128-wide contraction chunks
    n_xt = b_sh // P                  # x row tiles
    n_wt = w_sl // P                  # W row tiles
    n_jb = out_dim // NB              # output column blocks (== ncores)
    assert w_sl == NB and n_jb == ncores
    assert in_dim % (4 * P) == 0 and b_sh % (4 * P) == 0 and n_xt % 4 == 0
    nh = n_xt // 2                    # x tiles per half-pass
    bh = b_sh // 2                    # rows per half-pass

    nc = bacc.Bacc("TRN2", target_bir_lowering=False, debug=False,
                   num_devices=ncores)

    x_sh = nc.dram_tensor("x_sh", [b_sh, in_dim], F32, kind="ExternalInput")
    w_sl_t = nc.dram_tensor("w_sl", [w_sl, in_dim], F32, kind="ExternalInput")
    b2_rep = nc.dram_tensor("b2_rep", [P, out_dim], F32, kind="ExternalInput")
    ident_in = nc.dram_tensor("ident", [P, P], BF16, kind="ExternalInput")
    out_sh = nc.dram_tensor("out_sh", [b_sh, out_dim], F32,
                            kind="ExternalOutput")

    wqt_loc = nc.dram_tensor("wqt_loc", [in_dim, NB], BF16)
    wq_ag = nc.dram_tensor("wq_ag", [ncores * in_dim, NB], BF16,
                           addr_space="Shared")

    with tile.TileContext(nc) as tc:
        from contextlib import ExitStack
        with ExitStack() as ctx:
            qpool = ctx.enter_context(tc.tile_pool(name="qpool", bufs=2))
            spool = ctx.enter_context(tc.tile_pool(name="spool", bufs=2))
            big = ctx.enter_context(tc.tile_pool(name="big", bufs=1))
            wpool = ctx.enter_context(tc.tile_pool(name="wpool", bufs=16))
            opool = ctx.enter_context(tc.tile_pool(name="opool", bufs=3))
            mm_pool = ctx.enter_context(
                tc.tile_pool(name="mmp", bufs=2, space="PSUM"))

            ident = big.tile([P, P], BF16, tag="ident")
            nc.scalar.dma_start(ident[:], ident_in.ap())
            b2_sb = big.tile([P, out_dim], F32, tag="b2_sb")
            nc.scalar.dma_start(b2_sb[:], b2_rep.ap())

            # ---- quant phases (phase-split for software pipelining) --------
            # reduce/bits/clamp/stt are DVE-only ops; the big mult runs on
            # `mult_eng`; RNE rounding on the scalar engine.
            def q_load(src, row, tag, queue):
                xt = qpool.tile([P, in_dim], F32, tag=f"{tag}_xt", bufs=3)
                queue.dma_start(xt[:], src.ap()[row:row + P, :])
                return xt

            def q_scales(xt, tag):
                g = in_dim // SZ
                x3 = xt.rearrange("p (g s) -> p g s", s=SZ)
                m = spool.tile([P, g], F32, tag=f"{tag}_m")
                nc.vector.tensor_reduce(m[:], x3, axis=AX.X, op=ALU.max,
                                        apply_absolute_value=True)
                scale = spool.tile([P, g], F32, tag=f"{tag}_scale")
                nc.vector.tensor_scalar(
                    scale[:].bitcast(I32), m[:].bitcast(I32),
                    0x7F800000, None, op0=ALU.bitwise_and)
                nc.vector.tensor_scalar(
                    scale[:].bitcast(I32), scale[:].bitcast(I32),
                    7 << 23, None, op0=ALU.subtract)
                inv = spool.tile([P, g], F32, tag=f"{tag}_inv")
                nc.vector.tensor_scalar(
                    inv[:].bitcast(I32), scale[:].bitcast(I32),
                    -1, None, op0=ALU.bitwise_xor)
                nc.vector.tensor_scalar(
                    inv[:].bitcast(I32), inv[:].bitcast(I32),
                    (254 << 23) + 1, None, op0=ALU.add)
                return scale, inv

            def q_mult(xt, inv, eng):
                g = in_dim // SZ
                x3 = xt.rearrange("p (g s) -> p g s", s=SZ)
                eng.tensor_tensor(
                    x3, x3, inv[:, :, None].to_broadcast([P, g, SZ]), ALU.mult)

            def q_clamp(xt):
                nc.vector.tensor_scalar(
                    xt, xt, -128.25, 127.25, op0=ALU.max, op1=ALU.min)

            def q_round(xt):
                nc.scalar.activation(xt, xt, AF.Copy, bias=C_RND, scale=1.0)

            def q_stt(xt, scale, tag):
                g = in_dim // SZ
                q = qpool.tile([P, in_dim], BF16, tag=f"{tag}_q")
                nc.vector.scalar_tensor_tensor(
                    q[:].rearrange("p (g s) -> p g s", s=SZ),
                    xt.rearrange("p (g s) -> p g s", s=SZ),
                    C_RND,
                    scale[:, :, None].to_broadcast([P, g, SZ]),
                    op0=ALU.subtract, op1=ALU.mult)
                return q

            def quant_pair(src, rows, tag, queue, mult_eng):
                """Quantize two [P, in_dim] tiles, phase-interleaved.
                Returns the bf16 q tiles."""
                xts = [q_load(src, r, tag, queue) for r in rows]
                si = [q_scales(xt[:], tag) for xt in xts]
                for i, xt in enumerate(xts):
                    q_mult(xt[:], si[i][1][:], mult_eng)
                for xt in xts:
                    q_clamp(xt[:])
                for xt in xts:
                    q_round(xt[:])
                return [q_stt(xts[i][:], si[i][0][:], tag)
                        for i in range(len(xts))]

            # ---- W stage: quant + on-chip PE transpose + store -------------
            # per tile i: q [P, in_dim] -> 32 PE transposes -> wtt_i
            # [P, kc, P] -> one store into wqt_loc's column slice.
            def w_transpose_store(q, i):
                wtt = qpool.tile([P, kc, P], BF16, tag="wtt", bufs=2)
                for kq in range(kc // 4):
                    tp = mm_pool.tile([P, 4, P], BF16, tag=f"mm{kq % 2}",
                                      name=f"wtp_{i}_{kq}")
                    for t in range(4):
                        k = kq * 4 + t
                        nc.tensor.transpose(tp[:, t, :],
                                            q[:, k * P:(k + 1) * P], ident[:])
                    nc.vector.tensor_copy(wtt[:, kq * 4:(kq + 1) * 4, :],
                                          tp[:])
                # rows of wqt_loc iterate (k, p); columns slice i*P..(i+1)*P
                nc.scalar.dma_start(
                    wqt_loc.ap()[:, i * P:(i + 1) * P].rearrange(
                        "(k p) o -> p k o", p=P),
                    wtt[:])

            for t0 in range(0, n_wt, 2):
                qs = quant_pair(w_sl_t, [t0 * P, (t0 + 1) * P], "q",
                                nc.scalar, nc.gpsimd)
                for i, q in enumerate(qs):
                    w_transpose_store(q[:], t0 + i)

            def issue_ag():
                if for_timeline or ncores == 1:
                    nc.sync.dma_start(wq_ag.ap()[0:in_dim, :], wqt_loc.ap())
                else:
                    nc.gpsimd.collective_compute(
                        "AllGather", ALU.bypass,
                        replica_groups=[list(range(ncores))],
                        ins=[wqt_loc.ap().opt()],
                        outs=[wq_ag.ap().opt()])

            issue_ag()

            # ---- x quant + on-chip PE transpose into resident xqT.
            # The AllGather window is dead time for PE/DVE/Pool, so the
            # whole x stage runs inside it with no DRAM round-trip. --------
            xqT = big.tile([P, kc, b_sh], BF16, tag="xqT")

            def x_transpose(q, bb):
                for kq in range(kc // 4):
                    tp = mm_pool.tile([P, 4, P], BF16, tag=f"mm{kq % 2}",
                                      name=f"xtp_{bb}_{kq}")
                    for t in range(4):
                        k = kq * 4 + t
                        nc.tensor.transpose(tp[:, t, :],
                                            q[:, k * P:(k + 1) * P], ident[:])
                    nc.scalar.copy(
                        xqT[:, kq * 4:(kq + 1) * 4, bb * P:(bb + 1) * P],
                        tp[:])

            for t0 in range(0, n_xt, 2):
                qs = quant_pair(x_sh, [t0 * P, (t0 + 1) * P], "q",
                                nc.sync, nc.gpsimd)
                for i, q in enumerate(qs):
                    x_transpose(q[:], t0 + i)

            # ---- matmul passes ---------------------------------------------
            def drain(ps, bb, j):
                s = opool.tile([P, NB], F32, tag="ds")
                # scalar copy releases the PSUM bank promptly (DVE is busy)
                nc.scalar.copy(s[:], ps[:])
                nc.vector.tensor_tensor(s[:], s[:],
                                        b2_sb[:, j * NB:(j + 1) * NB],
                                        ALU.add)
                g = NB // SZ
                s3 = s[:].rearrange("p (g s) -> p g s", s=SZ)
                m = opool.tile([P, g], F32, tag="o_m")
                nc.vector.tensor_reduce(m[:], s3, axis=AX.X, op=ALU.max,
                                        apply_absolute_value=True)
                scale = opool.tile([P, g], F32, tag="o_scale")
                nc.vector.tensor_scalar(
                    scale[:].bitcast(I32), m[:].bitcast(I32),
                    0x7F800000, None, op0=ALU.bitwise_and)
                nc.vector.tensor_scalar(
                    scale[:].bitcast(I32), scale[:].bitcast(I32),
                    7 << 23, None, op0=ALU.subtract)
                inv = opool.tile([P, g], F32, tag="o_inv")
                nc.vector.tensor_scalar(
                    inv[:].bitcast(I32), scale[:].bitcast(I32),
                    -1, None, op0=ALU.bitwise_xor)
                nc.vector.tensor_scalar(
                    inv[:].bitcast(I32), inv[:].bitcast(I32),
                    (254 << 23) + 1, None, op0=ALU.add)
                nc.gpsimd.tensor_tensor(
                    s3, s3, inv[:, :, None].to_broadcast([P, g, SZ]),
                    ALU.mult)
                nc.vector.tensor_scalar(
                    s[:], s[:], -128.25, 127.25, op0=ALU.max, op1=ALU.min)
                nc.scalar.activation(s[:], s[:], AF.Copy, bias=C_RND,
                                     scale=1.0)
                oq = opool.tile([P, NB], F32, tag="oq")
                nc.vector.scalar_tensor_tensor(
                    oq[:].rearrange("p (g s) -> p g s", s=SZ),
                    s[:].rearrange("p (g s) -> p g s", s=SZ),
                    C_RND,
                    scale[:, :, None].to_broadcast([P, g, SZ]),
                    op0=ALU.subtract, op1=ALU.mult)
                nc.scalar.dma_start(
                    out_sh.ap()[bb * P:(bb + 1) * P, j * NB:(j + 1) * NB],
                    oq[:])

            def mm_pass(h):
                bbs = list(range(h * nh, (h + 1) * nh))
                for j in range(n_jb):
                    ps = [mm_pool.tile([P, NB], F32, tag=f"mm{i}",
                                       name=f"ps_{h}_{j}_{i}")
                          for i in range(len(bbs))]
                    for k in range(kc):
                        wqt = wpool.tile([P, NB], BF16, tag="wqt")
                        nc.sync.dma_start(
                            wqt[:],
                            wq_ag.ap()[j * in_dim + k * P:
                                       j * in_dim + (k + 1) * P, :])
                        for i, bb in enumerate(bbs):
                            nc.tensor.matmul(
                                ps[i][:],
                                lhsT=xqT[:, k, bb * P:(bb + 1) * P],
                                rhs=wqt[:],
                                start=(k == 0), stop=(k == kc - 1),
                                skip_group_check=True)
                    for i, bb in enumerate(bbs):
                        drain(ps[i], bb, j)

            mm_pass(0)
            mm_pass(1)

    nc.compile()
    return nc


_NC_CACHE = {}


def _get_nc(key=(B_FULL // NCORES, IN_FULL, OUT_FULL, NCORES)):
    if key not in _NC_CACHE:
        _NC_CACHE[key] = build_nc(*key)
    return _NC_CACHE[key]


def make_in_maps(x, W, b, ncores=NCORES):
    import ml_dtypes
    b_sh = x.shape[0] // ncores
    w_sl = W.shape[0] // ncores
    out_dim = W.shape[0]
    b2 = (2.0 * np.asarray(b, np.float32)).astype(np.float32)
    b2_rep = np.ascontiguousarray(
        np.broadcast_to(b2.reshape(1, out_dim), (P, out_dim)))
    ident = np.eye(P, dtype=ml_dtypes.bfloat16)
    return [
        {
            "x_sh": np.ascontiguousarray(x[c * b_sh:(c + 1) * b_sh]),
            "w_sl": np.ascontiguousarray(W[c * w_sl:(c + 1) * w_sl]),
            "b2_rep": b2_rep,
            "ident": ident,
        }
        for c in range(ncores)
    ]


def kernel(x, W, b):
    from concourse.bass_utils import run_bass_kernel_spmd

    x = np.asarray(x, np.float32)
    W = np.asarray(W, np.float32)
    b = np.asarray(b, np.float32)
    nc = _get_nc()
    in_maps = make_in_maps(x, W, b)
    res = run_bass_kernel_spmd(nc, in_maps, core_ids=list(range(NCORES)))
    return np.concatenate([res.results[c]["out_sh"] for c in range(NCORES)],
                          axis=0)



# revision 14
# speedup vs baseline: 1.1972x; 1.1972x over previous
"""BFLinear (block-floating-point quantized linear) Trainium2 kernel, v2.

Computes: out = bf_quant(bf_quant(x) @ bf_quant(W).T + 2*b)
where bf_quant quantizes groups of 32 along the last axis to a shared
power-of-two exponent with 8 mantissa bits (values = int8 * 2^(e-7)).

Distribution over 8 NeuronCores (SPMD, one identical program):
  - batch dim of x sharded 8 ways (1024 rows/core)
  - W quantization: rank r quantizes + PE-transposes W rows of output
    block r (contribution, AllGathered), and EVERY rank additionally
    quantizes block 7 locally from identical data ("w_sl2").  The
    matmul sweep processes the local block first (zero collective
    dependency), then gathered ranges 0..6.  Rank 7's gathered range
    is never read.  This keeps the program uniform across cores while
    giving each core ~250us of AllGather-independent work (x-quant,
    local W-quant, one matmul block) to hide the collective.
  - the AllGather is triggered from the Sync queue: the only
    instructions behind its wait are the post-AG weight-slab loads,
    which depend on the gathered data anyway.  A tiny warmup AllGather
    runs first to absorb collective-engine init / launch skew.
  - matmul: j-slab-outer, x-tile middle, k innermost => 32 consecutive
    matmuls accumulate into the SAME PSUM bank (avoids the PSUM
    bank-cycling HAM oscillation), weight slab fully SBUF-resident per
    block with one-ahead prefetch.

Quantization math (exact, matching jnp semantics):
  m     = max |x| over each group of 32          (abs-max reduce)
  scale = 2^(floor(log2 m) - 7)                  (exponent-field bit math)
  inv   = 1/scale                                (bit math, exact)
  r     = rne_round(clamp(x*inv)) via +C trick with C = 1.5*2^23
  q     = (r - C) * scale
"""

import numpy as np

# full-problem dimensions (hardcoded per harness contract)
B_FULL = 8192
IN_FULL = 4096
OUT_FULL = 4096
NCORES = 8

P = 128
SZ = 32
NB = 512                      # output column block width (= OUT/NCORES)
C_RND = float(3 * 2**22)      # 1.5*2^23: v+C in [2^23, 2^24) -> RNE to ints
LOCAL_BLK = NCORES - 1        # the block every core quantizes locally


def build_nc(b_sh=B_FULL // NCORES, in_dim=IN_FULL, out_dim=OUT_FULL,
             ncores=NCORES):
    """Build the SPMD Bass program (identical on every core; data differs)."""
    import concourse.mybir as mybir
    import concourse.tile as tile
    from concourse import bacc

    F32 = mybir.dt.float32
    BF16 = mybir.dt.bfloat16
    I32 = mybir.dt.int32
    ALU = mybir.AluOpType
    AX = mybir.AxisListType
    AF = mybir.ActivationFunctionType

    w_sl = out_dim // ncores          # W rows per block
    kc = in_dim // P                  # 128-wide contraction chunks
    n_xt = b_sh // P                  # x row tiles
    n_wt = w_sl // P                  # W row tiles per block
    assert w_sl == NB
    assert in_dim % (4 * P) == 0 and b_sh % P == 0

    nc = bacc.Bacc("TRN2", target_bir_lowering=False, debug=False,
                   num_devices=ncores)

    x_sh = nc.dram_tensor("x_sh", [b_sh, in_dim], F32, kind="ExternalInput")
    w_sl_t = nc.dram_tensor("w_sl", [w_sl, in_dim], F32, kind="ExternalInput")
    w_sl2_t = nc.dram_tensor("w_sl2", [w_sl, in_dim], F32,
                             kind="ExternalInput")
    b2_rep = nc.dram_tensor("b2_rep", [P, out_dim], F32, kind="ExternalInput")
    ident_in = nc.dram_tensor("ident", [P, P], BF16, kind="ExternalInput")
    out_sh = nc.dram_tensor("out_sh", [b_sh, out_dim], F32,
                            kind="ExternalOutput")

    wqt_loc = nc.dram_tensor("wqt_loc", [in_dim, NB], BF16)
    wq_ag = nc.dram_tensor("wq_ag", [ncores * in_dim, NB], BF16,
                           addr_space="Shared")
    # warmup-collective scratch: contents never read, only the rendezvous
    # side effect matters (collectives cannot touch IO tensors)
    warm_src = nc.dram_tensor("warm_src", [P, 8], BF16)
    warm_ag = nc.dram_tensor("warm_ag", [ncores * P, 8], BF16,
                             addr_space="Shared")

    with tile.TileContext(nc) as tc:
        from contextlib import ExitStack
        with ExitStack() as ctx:
            qpool = ctx.enter_context(tc.tile_pool(name="qpool", bufs=2))
            spool = ctx.enter_context(tc.tile_pool(name="spool", bufs=2))
            big = ctx.enter_context(tc.tile_pool(name="big", bufs=1))
            wpool = ctx.enter_context(tc.tile_pool(name="wpool", bufs=2))
            opool = ctx.enter_context(tc.tile_pool(name="opool", bufs=3))
            mm_pool = ctx.enter_context(
                tc.tile_pool(name="mmp", bufs=2, space="PSUM"))

            ident = big.tile([P, P], BF16, tag="ident")
            nc.scalar.dma_start(ident[:], ident_in.ap())
            b2_sb = big.tile([P, out_dim], F32, tag="b2_sb")
            nc.scalar.dma_start(b2_sb[:], b2_rep.ap())

            # tiny warmup collective: rendezvous the 8 cores + pay the
            # collective-engine init cost early, overlapped with quant.
            # collectives only exist on gpsimd, so gpsimd carries ONLY the
            # two collectives and post-AG drain multiplies: nothing local
            # ever queues behind a collective wait.
            nc.gpsimd.collective_compute(
                "AllGather", ALU.bypass,
                replica_groups=[list(range(ncores))],
                ins=[warm_src.ap().opt()],
                outs=[warm_ag.ap().opt()])

            # ---- quant phases (phase-split for software pipelining) --------
            def q_load(src, row, tag, queue):
                xt = qpool.tile([P, in_dim], F32, tag="xt", name=f"xt_{tag}")
                queue.dma_start(xt[:], src.ap()[row:row + P, :])
                return xt

            def q_scales(xt, tag):
                g = in_dim // SZ
                x3 = xt.rearrange("p (g s) -> p g s", s=SZ)
                m = spool.tile([P, g], F32, tag="q_m", name=f"m_{tag}")
                nc.vector.tensor_reduce(m[:], x3, axis=AX.X, op=ALU.max,
                                        apply_absolute_value=True)
                scale = spool.tile([P, g], F32, tag="q_scale",
                                   name=f"scale_{tag}")
                nc.vector.tensor_scalar(
                    scale[:].bitcast(I32), m[:].bitcast(I32),
                    0x7F800000, None, op0=ALU.bitwise_and)
                nc.vector.tensor_scalar(
                    scale[:].bitcast(I32), scale[:].bitcast(I32),
                    7 << 23, None, op0=ALU.subtract)
                inv = spool.tile([P, g], F32, tag="q_inv", name=f"inv_{tag}")
                nc.vector.tensor_scalar(
                    inv[:].bitcast(I32), scale[:].bitcast(I32),
                    -1, None, op0=ALU.bitwise_xor)
                nc.vector.tensor_scalar(
                    inv[:].bitcast(I32), inv[:].bitcast(I32),
                    (254 << 23) + 1, None, op0=ALU.add)
                return scale, inv

            def q_mult(xt, inv):
                g = in_dim // SZ
                x3 = xt.rearrange("p (g s) -> p g s", s=SZ)
                nc.vector.tensor_tensor(
                    x3, x3, inv[:, :, None].to_broadcast([P, g, SZ]), ALU.mult)

            def q_clamp(xt):
                nc.vector.tensor_scalar(
                    xt, xt, -128.25, 127.25, op0=ALU.max, op1=ALU.min)

            def q_round(xt):
                nc.scalar.activation(xt, xt, AF.Copy, bias=C_RND, scale=1.0)

            def q_stt(xt, scale, tag):
                g = in_dim // SZ
                q = qpool.tile([P, in_dim], BF16, tag="q", bufs=2,
                               name=f"q_{tag}")
                nc.vector.scalar_tensor_tensor(
                    q[:].rearrange("p (g s) -> p g s", s=SZ),
                    xt.rearrange("p (g s) -> p g s", s=SZ),
                    C_RND,
                    scale[:, :, None].to_broadcast([P, g, SZ]),
                    op0=ALU.subtract, op1=ALU.mult)
                return q

            def quant_pair(src, rows, tag, queue):
                """Quantize two [P, in_dim] tiles, phase-interleaved."""
                xts = [q_load(src, r, f"{tag}{i}", queue)
                       for i, r in enumerate(rows)]
                si = [q_scales(xt[:], f"{tag}{i}")
                      for i, xt in enumerate(xts)]
                for i, xt in enumerate(xts):
                    q_mult(xt[:], si[i][1][:])
                for xt in xts:
                    q_clamp(xt[:])
                for xt in xts:
                    q_round(xt[:])
                return [q_stt(xts[i][:], si[i][0][:], f"{tag}{i}")
                        for i in range(len(xts))]

            # ---- W contribution block: quant + PE transpose + store + AG ---
            def w_transpose_store(q, i):
                wtt = wpool.tile([P, kc, P], BF16, tag="w", name=f"wtt_{i}")
                for kq in range(kc // 4):
                    tp = mm_pool.tile([P, 4, P], BF16, tag="tp",
                                      name=f"wtp_{i}_{kq}")
                    for t in range(4):
                        k = kq * 4 + t
                        nc.tensor.transpose(tp[:, t, :],
                                            q[:, k * P:(k + 1) * P], ident[:])
                    nc.vector.tensor_copy(wtt[:, kq * 4:(kq + 1) * 4, :],
                                          tp[:])
                nc.scalar.dma_start(
                    wqt_loc.ap()[:, i * P:(i + 1) * P].rearrange(
                        "(k p) o -> p k o", p=P),
                    wtt[:])

            for t0 in range(0, n_wt, 2):
                qs = quant_pair(w_sl_t, [t0 * P, (t0 + 1) * P], "wc",
                                nc.scalar)
                for i, q in enumerate(qs):
                    w_transpose_store(q[:], t0 + i)

            # main AllGather (gpsimd carries only collectives + post-AG work)
            nc.gpsimd.collective_compute(
                "AllGather", ALU.bypass,
                replica_groups=[list(range(ncores))],
                ins=[wqt_loc.ap().opt()],
                outs=[wq_ag.ap().opt()])

            # ---- local W block (block 7, identical on every core) ----------
            # transposed straight into an SBUF-resident slab: no DRAM
            # round-trip, no collective dependency.
            slabL = wpool.tile([P, kc, NB], BF16, tag="w", name="slabL")

            def w_transpose_local(q, i):
                for kq in range(kc // 4):
                    tp = mm_pool.tile([P, 4, P], BF16, tag="tp",
                                      name=f"ltp_{i}_{kq}")
                    for t in range(4):
                        k = kq * 4 + t
                        nc.tensor.transpose(tp[:, t, :],
                                            q[:, k * P:(k + 1) * P], ident[:])
                    nc.vector.tensor_copy(
                        slabL[:, kq * 4:(kq + 1) * 4, i * P:(i + 1) * P],
                        tp[:])

            for t0 in range(0, n_wt, 2):
                qs = quant_pair(w_sl2_t, [t0 * P, (t0 + 1) * P], "wl",
                                nc.scalar)
                for i, q in enumerate(qs):
                    w_transpose_local(q[:], t0 + i)

            # ---- x quant + PE transpose into resident xqT ------------------
            xqT = big.tile([P, kc, b_sh], BF16, tag="xqT")

            def x_transpose(q, bb):
                for kq in range(kc // 4):
                    tp = mm_pool.tile([P, 4, P], BF16, tag="tp",
                                      name=f"xtp_{bb}_{kq}")
                    for t in range(4):
                        k = kq * 4 + t
                        nc.tensor.transpose(tp[:, t, :],
                                            q[:, k * P:(k + 1) * P], ident[:])
                    nc.scalar.copy(
                        xqT[:, kq * 4:(kq + 1) * 4, bb * P:(bb + 1) * P],
                        tp[:])

            for t0 in range(0, n_xt, 2):
                qs = quant_pair(x_sh, [t0 * P, (t0 + 1) * P], "x",
                                nc.sync)
                for i, q in enumerate(qs):
                    x_transpose(q[:], t0 + i)

            # ---- matmul sweep ----------------------------------------------
            def drain(ps, bb, jcol, mult_eng):
                s = opool.tile([P, NB], F32, tag="ds", name=f"s_{jcol}_{bb}")
                nc.scalar.copy(s[:], ps[:])   # releases the PSUM bank
                nc.vector.tensor_tensor(
                    s[:], s[:], b2_sb[:, jcol * NB:(jcol + 1) * NB], ALU.add)
                g = NB // SZ
                s3 = s[:].rearrange("p (g s) -> p g s", s=SZ)
                m = opool.tile([P, g], F32, tag="o_m", name=f"om_{jcol}_{bb}")
                nc.vector.tensor_reduce(m[:], s3, axis=AX.X, op=ALU.max,
                                        apply_absolute_value=True)
                scale = opool.tile([P, g], F32, tag="o_scale",
                                   name=f"osc_{jcol}_{bb}")
                nc.vector.tensor_scalar(
                    scale[:].bitcast(I32), m[:].bitcast(I32),
                    0x7F800000, None, op0=ALU.bitwise_and)
                nc.vector.tensor_scalar(
                    scale[:].bitcast(I32), scale[:].bitcast(I32),
                    7 << 23, None, op0=ALU.subtract)
                inv = opool.tile([P, g], F32, tag="o_inv",
                                 name=f"oin_{jcol}_{bb}")
                nc.vector.tensor_scalar(
                    inv[:].bitcast(I32), scale[:].bitcast(I32),
                    -1, None, op0=ALU.bitwise_xor)
                nc.vector.tensor_scalar(
                    inv[:].bitcast(I32), inv[:].bitcast(I32),
                    (254 << 23) + 1, None, op0=ALU.add)
                mult_eng.tensor_tensor(
                    s3, s3, inv[:, :, None].to_broadcast([P, g, SZ]),
                    ALU.mult)
                nc.vector.tensor_scalar(
                    s[:], s[:], -128.25, 127.25, op0=ALU.max, op1=ALU.min)
                nc.scalar.activation(s[:], s[:], AF.Copy, bias=C_RND,
                                     scale=1.0)
                oq = opool.tile([P, NB], F32, tag="oq",
                                name=f"oq_{jcol}_{bb}")
                nc.vector.scalar_tensor_tensor(
                    oq[:].rearrange("p (g s) -> p g s", s=SZ),
                    s[:].rearrange("p (g s) -> p g s", s=SZ),
                    C_RND,
                    scale[:, :, None].to_broadcast([P, g, SZ]),
                    op0=ALU.subtract, op1=ALU.mult)
                nc.scalar.dma_start(
                    out_sh.ap()[bb * P:(bb + 1) * P,
                                jcol * NB:(jcol + 1) * NB],
                    oq[:])

            for t in range(ncores):
                if t == 0:
                    slab = slabL
                    jcol = LOCAL_BLK
                else:
                    jcol = t - 1
                    slab = wpool.tile([P, kc, NB], BF16, tag="w",
                                      name=f"slab_{t}")
                    nc.sync.dma_start(
                        slab[:],
                        wq_ag.ap()[jcol * in_dim:(jcol + 1) * in_dim, :]
                        .rearrange("(k p) o -> p k o", p=P))
                for bb in range(n_xt):
                    ps = mm_pool.tile([P, NB], F32, tag="ps", bufs=3,
                                      name=f"ps_{t}_{bb}")
                    for k in range(kc):
                        nc.tensor.matmul(
                            ps[:],
                            lhsT=xqT[:, k, bb * P:(bb + 1) * P],
                            rhs=slab[:, k, :],
                            start=(k == 0), stop=(k == kc - 1),
                            skip_group_check=True)
                    # t=0 drains run pre-AG: keep them off the gpsimd
                    # queue (which is waiting on the AllGather)
                    drain(ps, bb, jcol,
                          nc.vector if t == 0 else nc.gpsimd)

    nc.compile()
    return nc


_NC_CACHE = {}


def _get_nc(key=(B_FULL // NCORES, IN_FULL, OUT_FULL, NCORES)):
    if key not in _NC_CACHE:
        _NC_CACHE[key] = build_nc(*key)
    return _NC_CACHE[key]


def make_in_maps(x, W, b, ncores=NCORES):
    import ml_dtypes
    b_sh = x.shape[0] // ncores
    w_sl = W.shape[0] // ncores
    out_dim = W.shape[0]
    b2 = (2.0 * np.asarray(b, np.float32)).astype(np.float32)
    b2_rep = np.ascontiguousarray(
        np.broadcast_to(b2.reshape(1, out_dim), (P, out_dim)))
    ident = np.eye(P, dtype=ml_dtypes.bfloat16)
    w_local = np.ascontiguousarray(
        W[LOCAL_BLK * w_sl:(LOCAL_BLK + 1) * w_sl])
    return [
        {
            "x_sh": np.ascontiguousarray(x[c * b_sh:(c + 1) * b_sh]),
            "w_sl": np.ascontiguousarray(W[c * w_sl:(c + 1) * w_sl]),
            "w_sl2": w_local,
            "b2_rep": b2_rep,
            "ident": ident,
        }
        for c in range(ncores)
    ]


def kernel(x, W, b):
    from concourse.bass_utils import run_bass_kernel_spmd

    x = np.asarray(x, np.float32)
    W = np.asarray(W, np.float32)
    b = np.asarray(b, np.float32)
    nc = _get_nc()
    in_maps = make_in_maps(x, W, b)
    res = run_bass_kernel_spmd(nc, in_maps, core_ids=list(range(NCORES)))
    return np.concatenate([res.results[c]["out_sh"] for c in range(NCORES)],
                          axis=0)


# revision 15
# speedup vs baseline: 1.1984x; 1.0010x over previous
"""BFLinear (block-floating-point quantized linear) Trainium2 kernel, v3.

Computes: out = bf_quant(bf_quant(x) @ bf_quant(W).T + 2*b)
where bf_quant quantizes groups of 32 along the last axis to a shared
power-of-two exponent with 8 mantissa bits (values = int8 * 2^(e-7)).

Distribution over 8 NeuronCores (SPMD, one identical program):
  - batch dim of x sharded 8 ways (1024 rows/core)
  - W quantization: rank r quantizes + PE-transposes W rows of output
    block r (contribution, AllGathered), and EVERY rank additionally
    quantizes block 7 locally from identical data ("w_sl2").  The
    matmul sweep processes the local block first (zero collective
    dependency), then gathered ranges 0..6.  Rank 7's gathered range
    is never read.  Uniform program, per-core data.
  - a tiny warmup AllGather runs first (absorbs launch skew +
    collective init); the real AllGather is triggered as soon as the
    contribution block is stored, and completes under the local
    matmul work (x-quant + block-7 matmul).
  - matmul sweep: slab-outer, x-tile middle, k innermost: 32
    consecutive matmuls accumulate into one PSUM bank; weight slabs
    are SBUF-resident with one-ahead prefetch.  Measured rate is the
    power-throttled PE ceiling (K=13/16), so the sweep is PE-bound.
  - quantization runs on half-width tiles ([128, 2048]) for pipeline
    latency; round+clamp is a single scalar-engine saturating int8
    convert, dequant is one DVE multiply (int8 x scale -> bf16).

Quantization math (matching jnp semantics):
  m     = max |x| over each group of 32          (abs-max reduce)
  scale = 2^(floor(log2 m) - 7)                  (exponent-field bit math)
  inv   = 1/scale                                (bit math, exact)
  i8    = sat_int8(rne(x*inv))                   (ACT convert)
  q     = i8 * scale                             (exact in bf16)
"""

import numpy as np

# full-problem dimensions (hardcoded per harness contract)
B_FULL = 8192
IN_FULL = 4096
OUT_FULL = 4096
NCORES = 8

P = 128
SZ = 32
NB = 512                      # output column block width (= OUT/NCORES)
HW = 2048                     # quant half-tile width
LOCAL_BLK = NCORES - 1        # the block every core quantizes locally


def build_nc(b_sh=B_FULL // NCORES, in_dim=IN_FULL, out_dim=OUT_FULL,
             ncores=NCORES):
    """Build the SPMD Bass program (identical on every core; data differs)."""
    import concourse.mybir as mybir
    import concourse.tile as tile
    from concourse import bacc

    F32 = mybir.dt.float32
    BF16 = mybir.dt.bfloat16
    I32 = mybir.dt.int32
    I8 = mybir.dt.int8
    ALU = mybir.AluOpType
    AX = mybir.AxisListType
    AF = mybir.ActivationFunctionType

    w_sl = out_dim // ncores          # W rows per block
    kc = in_dim // P                  # 128-wide contraction chunks
    n_xt = b_sh // P                  # x row tiles
    n_wt = w_sl // P                  # W row tiles per block
    kh = HW // P                      # k-chunks per half tile (16)
    assert w_sl == NB and in_dim == 2 * HW

    nc = bacc.Bacc("TRN2", target_bir_lowering=False, debug=False,
                   num_devices=ncores)

    x_sh = nc.dram_tensor("x_sh", [b_sh, in_dim], F32, kind="ExternalInput")
    w_sl_t = nc.dram_tensor("w_sl", [w_sl, in_dim], F32, kind="ExternalInput")
    w_sl2_t = nc.dram_tensor("w_sl2", [w_sl, in_dim], F32,
                             kind="ExternalInput")
    b2_rep = nc.dram_tensor("b2_rep", [P, out_dim], F32, kind="ExternalInput")
    ident_in = nc.dram_tensor("ident", [P, P], BF16, kind="ExternalInput")
    out_sh = nc.dram_tensor("out_sh", [b_sh, out_dim], F32,
                            kind="ExternalOutput")

    wqt_loc = nc.dram_tensor("wqt_loc", [in_dim, NB], BF16)
    wq_ag = nc.dram_tensor("wq_ag", [ncores * in_dim, NB], BF16,
                           addr_space="Shared")
    # warmup-collective scratch: contents never read, only the rendezvous
    # side effect matters (collectives cannot touch IO tensors)
    warm_src = nc.dram_tensor("warm_src", [P, 8], BF16)
    warm_ag = nc.dram_tensor("warm_ag", [ncores * P, 8], BF16,
                             addr_space="Shared")

    with tile.TileContext(nc) as tc:
        from contextlib import ExitStack
        with ExitStack() as ctx:
            qpool = ctx.enter_context(tc.tile_pool(name="qpool", bufs=3))
            spool = ctx.enter_context(tc.tile_pool(name="spool", bufs=3))
            big = ctx.enter_context(tc.tile_pool(name="big", bufs=1))
            wpool = ctx.enter_context(tc.tile_pool(name="wpool", bufs=2))
            opool = ctx.enter_context(tc.tile_pool(name="opool", bufs=3))
            mm_pool = ctx.enter_context(
                tc.tile_pool(name="mmp", bufs=2, space="PSUM"))

            ident = big.tile([P, P], BF16, tag="ident")
            nc.scalar.dma_start(ident[:], ident_in.ap())
            b2_sb = big.tile([P, out_dim], F32, tag="b2_sb")
            nc.scalar.dma_start(b2_sb[:], b2_rep.ap())

            # tiny warmup collective (gpsimd carries only the collectives
            # and post-AG drain multiplies, so nothing local queues behind
            # a collective wait)
            nc.gpsimd.collective_compute(
                "AllGather", ALU.bypass,
                replica_groups=[list(range(ncores))],
                ins=[warm_src.ap().opt()],
                outs=[warm_ag.ap().opt()])

            # ---- quant: half-tiles, phase-split over a pair --------------
            def q_half(src, row, col, tag, queue):
                """Load + start quant chain for one [P, HW] half tile.
                Returns (i8, scale) for the dequant step."""
                xt = qpool.tile([P, HW], F32, tag="xt", name=f"xt_{tag}")
                queue.dma_start(xt[:], src.ap()[row:row + P, col:col + HW])
                return xt

            def q_scales(xt, tag):
                g = HW // SZ
                x3 = xt.rearrange("p (g s) -> p g s", s=SZ)
                m = spool.tile([P, g], F32, tag="q_m", name=f"m_{tag}")
                nc.vector.tensor_reduce(m[:], x3, axis=AX.X, op=ALU.max,
                                        apply_absolute_value=True)
                scale = spool.tile([P, g], F32, tag="q_scale",
                                   name=f"scale_{tag}")
                nc.vector.tensor_scalar(
                    scale[:].bitcast(I32), m[:].bitcast(I32),
                    0x7F800000, None, op0=ALU.bitwise_and)
                nc.vector.tensor_scalar(
                    scale[:].bitcast(I32), scale[:].bitcast(I32),
                    7 << 23, None, op0=ALU.subtract)
                inv = spool.tile([P, g], F32, tag="q_inv", name=f"inv_{tag}")
                nc.vector.tensor_scalar(
                    inv[:].bitcast(I32), scale[:].bitcast(I32),
                    -1, None, op0=ALU.bitwise_xor)
                nc.vector.tensor_scalar(
                    inv[:].bitcast(I32), inv[:].bitcast(I32),
                    (254 << 23) + 1, None, op0=ALU.add)
                return scale, inv

            def q_mult(xt, inv):
                g = HW // SZ
                x3 = xt.rearrange("p (g s) -> p g s", s=SZ)
                nc.vector.tensor_tensor(
                    x3, x3, inv[:, :, None].to_broadcast([P, g, SZ]), ALU.mult)

            def q_int8(xt, tag):
                i8 = qpool.tile([P, HW], I8, tag="i8", bufs=2,
                                name=f"i8_{tag}")
                nc.scalar.activation(i8[:], xt, AF.Copy, bias=0.0, scale=1.0)
                return i8

            def q_deq(i8, scale, tag):
                g = HW // SZ
                q = qpool.tile([P, HW], BF16, tag="q", bufs=2,
                               name=f"q_{tag}")
                nc.vector.tensor_tensor(
                    q[:].rearrange("p (g s) -> p g s", s=SZ),
                    i8.rearrange("p (g s) -> p g s", s=SZ),
                    scale[:, :, None].to_broadcast([P, g, SZ]), ALU.mult)
                return q

            def quant_pair(src, rowcols, tag, queue):
                """Quantize two [P, HW] half tiles, phase-interleaved.
                Returns the bf16 q half-tiles."""
                xts = [q_half(src, r, c, f"{tag}{i}", queue)
                       for i, (r, c) in enumerate(rowcols)]
                si = [q_scales(xt[:], f"{tag}{i}")
                      for i, xt in enumerate(xts)]
                for i, xt in enumerate(xts):
                    q_mult(xt[:], si[i][1][:])
                i8s = [q_int8(xts[i][:], f"{tag}{i}") for i in range(2)]
                return [q_deq(i8s[i][:], si[i][0][:], f"{tag}{i}")
                        for i in range(2)]

            # transpose one bf16 q half-tile (16 k-chunks) into `sink`:
            # sink[:, k0+kq*4:(kq+1)*4, cc:cc+P] gets the transposed chunks
            def half_transpose(q, k0, sink, cc, tag, copy_eng):
                for kq in range(kh // 4):
                    tp = mm_pool.tile([P, 4, P], BF16, tag="tp", bufs=4,
                                      name=f"tp_{tag}_{kq}")
                    for t in range(4):
                        k = kq * 4 + t
                        nc.tensor.transpose(tp[:, t, :],
                                            q[:, k * P:(k + 1) * P], ident[:])
                    copy_eng.copy(
                        sink[:, k0 + kq * 4:k0 + (kq + 1) * 4, cc:cc + P],
                        tp[:])

            # ---- W contribution block: quant + transpose + store + AG ----
            for i in range(n_wt):
                qs = quant_pair(w_sl_t, [(i * P, 0), (i * P, HW)], f"wc{i}",
                                nc.scalar)
                wtt = wpool.tile([P, kc, P], BF16, tag="w", name=f"wtt_{i}")
                for h, q in enumerate(qs):
                    half_transpose(q[:], h * kh, wtt, 0, f"wc{i}{h}",
                                   nc.scalar)
                nc.scalar.dma_start(
                    wqt_loc.ap()[:, i * P:(i + 1) * P].rearrange(
                        "(k p) o -> p k o", p=P),
                    wtt[:, :, 0:P])

            # main AllGather
            nc.gpsimd.collective_compute(
                "AllGather", ALU.bypass,
                replica_groups=[list(range(ncores))],
                ins=[wqt_loc.ap().opt()],
                outs=[wq_ag.ap().opt()])

            # ---- local W block (block 7, identical on every core) --------
            slabL = wpool.tile([P, kc, NB], BF16, tag="w", name="slabL")
            for i in range(n_wt):
                qs = quant_pair(w_sl2_t, [(i * P, 0), (i * P, HW)], f"wl{i}",
                                nc.scalar)
                for h, q in enumerate(qs):
                    half_transpose(q[:], h * kh, slabL, i * P, f"wl{i}{h}",
                                   nc.scalar)

            # ---- x quant + transpose into resident xqT -------------------
            xqT = big.tile([P, kc, b_sh], BF16, tag="xqT")
            for bb in range(n_xt):
                qs = quant_pair(x_sh, [(bb * P, 0), (bb * P, HW)], f"x{bb}",
                                nc.sync)
                for h, q in enumerate(qs):
                    half_transpose(q[:], h * kh, xqT, bb * P, f"x{bb}{h}",
                                   nc.scalar)

            # ---- matmul sweep --------------------------------------------
            def drain(ps, bb, jcol, mult_eng):
                s = opool.tile([P, NB], F32, tag="ds", name=f"s_{jcol}_{bb}")
                nc.scalar.copy(s[:], ps[:])   # releases the PSUM bank
                nc.vector.tensor_tensor(
                    s[:], s[:], b2_sb[:, jcol * NB:(jcol + 1) * NB], ALU.add)
                g = NB // SZ
                s3 = s[:].rearrange("p (g s) -> p g s", s=SZ)
                m = opool.tile([P, g], F32, tag="o_m", name=f"om_{jcol}_{bb}")
                nc.vector.tensor_reduce(m[:], s3, axis=AX.X, op=ALU.max,
                                        apply_absolute_value=True)
                scale = opool.tile([P, g], F32, tag="o_scale",
                                   name=f"osc_{jcol}_{bb}")
                nc.vector.tensor_scalar(
                    scale[:].bitcast(I32), m[:].bitcast(I32),
                    0x7F800000, None, op0=ALU.bitwise_and)
                nc.vector.tensor_scalar(
                    scale[:].bitcast(I32), scale[:].bitcast(I32),
                    7 << 23, None, op0=ALU.subtract)
                inv = opool.tile([P, g], F32, tag="o_inv",
                                 name=f"oin_{jcol}_{bb}")
                nc.vector.tensor_scalar(
                    inv[:].bitcast(I32), scale[:].bitcast(I32),
                    -1, None, op0=ALU.bitwise_xor)
                nc.vector.tensor_scalar(
                    inv[:].bitcast(I32), inv[:].bitcast(I32),
                    (254 << 23) + 1, None, op0=ALU.add)
                mult_eng.tensor_tensor(
                    s3, s3, inv[:, :, None].to_broadcast([P, g, SZ]),
                    ALU.mult)
                oi8 = opool.tile([P, NB], I8, tag="oi8",
                                 name=f"oi8_{jcol}_{bb}")
                nc.scalar.activation(oi8[:], s[:], AF.Copy, bias=0.0,
                                     scale=1.0)
                oq = opool.tile([P, NB], F32, tag="oq",
                                name=f"oq_{jcol}_{bb}")
                nc.vector.tensor_tensor(
                    oq[:].rearrange("p (g s) -> p g s", s=SZ),
                    oi8[:].rearrange("p (g s) -> p g s", s=SZ),
                    scale[:, :, None].to_broadcast([P, g, SZ]), ALU.mult)
                nc.scalar.dma_start(
                    out_sh.ap()[bb * P:(bb + 1) * P,
                                jcol * NB:(jcol + 1) * NB],
                    oq[:])

            for t in range(ncores):
                if t == 0:
                    slab = slabL
                    jcol = LOCAL_BLK
                else:
                    jcol = t - 1
                    slab = wpool.tile([P, kc, NB], BF16, tag="w",
                                      name=f"slab_{t}")
                    nc.sync.dma_start(
                        slab[:],
                        wq_ag.ap()[jcol * in_dim:(jcol + 1) * in_dim, :]
                        .rearrange("(k p) o -> p k o", p=P))
                for bb in range(n_xt):
                    ps = mm_pool.tile([P, NB], F32, tag="ps", bufs=3,
                                      name=f"ps_{t}_{bb}")
                    for k in range(kc):
                        nc.tensor.matmul(
                            ps[:],
                            lhsT=xqT[:, k, bb * P:(bb + 1) * P],
                            rhs=slab[:, k, :],
                            start=(k == 0), stop=(k == kc - 1),
                            skip_group_check=True)
                    # t=0 drains run pre-AG: keep them off the gpsimd
                    # queue (which is waiting on the AllGather)
                    drain(ps, bb, jcol,
                          nc.vector if t == 0 else nc.gpsimd)

    nc.compile()
    return nc


_NC_CACHE = {}


def _get_nc(key=(B_FULL // NCORES, IN_FULL, OUT_FULL, NCORES)):
    if key not in _NC_CACHE:
        _NC_CACHE[key] = build_nc(*key)
    return _NC_CACHE[key]


def make_in_maps(x, W, b, ncores=NCORES):
    import ml_dtypes
    b_sh = x.shape[0] // ncores
    w_sl = W.shape[0] // ncores
    out_dim = W.shape[0]
    b2 = (2.0 * np.asarray(b, np.float32)).astype(np.float32)
    b2_rep = np.ascontiguousarray(
        np.broadcast_to(b2.reshape(1, out_dim), (P, out_dim)))
    ident = np.eye(P, dtype=ml_dtypes.bfloat16)
    w_local = np.ascontiguousarray(
        W[LOCAL_BLK * w_sl:(LOCAL_BLK + 1) * w_sl])
    return [
        {
            "x_sh": np.ascontiguousarray(x[c * b_sh:(c + 1) * b_sh]),
            "w_sl": np.ascontiguousarray(W[c * w_sl:(c + 1) * w_sl]),
            "w_sl2": w_local,
            "b2_rep": b2_rep,
            "ident": ident,
        }
        for c in range(ncores)
    ]


def kernel(x, W, b):
    from concourse.bass_utils import run_bass_kernel_spmd

    x = np.asarray(x, np.float32)
    W = np.asarray(W, np.float32)
    b = np.asarray(b, np.float32)
    nc = _get_nc()
    in_maps = make_in_maps(x, W, b)
    res = run_bass_kernel_spmd(nc, in_maps, core_ids=list(range(NCORES)))
    return np.concatenate([res.results[c]["out_sh"] for c in range(NCORES)],
                          axis=0)


# revision 18
# speedup vs baseline: 1.3397x; 1.1179x over previous
"""BFLinear (block-floating-point quantized linear) Trainium2 kernel, v3.

Computes: out = bf_quant(bf_quant(x) @ bf_quant(W).T + 2*b)
where bf_quant quantizes groups of 32 along the last axis to a shared
power-of-two exponent with 8 mantissa bits (values = int8 * 2^(e-7)).

Distribution over 8 NeuronCores (SPMD, one identical program):
  - batch dim of x sharded 8 ways (1024 rows/core)
  - W quantization: rank r quantizes + PE-transposes W rows of output
    block r (contribution, AllGathered), and EVERY rank additionally
    quantizes block 7 locally from identical data ("w_sl2").  The
    matmul sweep processes the local block first (zero collective
    dependency), then gathered ranges 0..6.  Rank 7's gathered range
    is never read.  Uniform program, per-core data.
  - a tiny warmup AllGather runs first (absorbs launch skew +
    collective init); the real AllGather is triggered as soon as the
    contribution block is stored, and completes under the local
    matmul work (x-quant + block-7 matmul).
  - matmul sweep: slab-outer, x-tile middle, k innermost: 32
    consecutive matmuls accumulate into one PSUM bank; weight slabs
    are SBUF-resident with one-ahead prefetch.  Measured rate is the
    power-throttled PE ceiling (K=13/16), so the sweep is PE-bound.
  - quantization runs on half-width tiles ([128, 2048]) for pipeline
    latency; round+clamp is a single scalar-engine saturating int8
    convert, dequant is one DVE multiply (int8 x scale -> bf16).

Quantization math (matching jnp semantics):
  m     = max |x| over each group of 32          (abs-max reduce)
  scale = 2^(floor(log2 m) - 7)                  (exponent-field bit math)
  inv   = 1/scale                                (bit math, exact)
  i8    = sat_int8(rne(x*inv))                   (ACT convert)
  q     = i8 * scale                             (exact in bf16)
"""

import numpy as np

# full-problem dimensions (hardcoded per harness contract)
B_FULL = 8192
IN_FULL = 4096
OUT_FULL = 4096
NCORES = 8

P = 128
SZ = 32
NB = 512                      # output column block width (= OUT/NCORES)
HW = 2048                     # quant half-tile width
LOCAL_BLK = NCORES - 1        # the block every core quantizes locally


def build_nc(b_sh=B_FULL // NCORES, in_dim=IN_FULL, out_dim=OUT_FULL,
             ncores=NCORES):
    """Build the SPMD Bass program (identical on every core; data differs)."""
    import concourse.mybir as mybir
    import concourse.tile as tile
    from concourse import bacc

    F32 = mybir.dt.float32
    BF16 = mybir.dt.bfloat16
    I32 = mybir.dt.int32
    I8 = mybir.dt.int8
    ALU = mybir.AluOpType
    AX = mybir.AxisListType
    AF = mybir.ActivationFunctionType

    w_sl = out_dim // ncores          # W rows per block
    kc = in_dim // P                  # 128-wide contraction chunks
    n_xt = b_sh // P                  # x row tiles
    n_wt = w_sl // P                  # W row tiles per block
    kh = HW // P                      # k-chunks per half tile (16)
    assert w_sl == NB and in_dim == 2 * HW

    nc = bacc.Bacc("TRN2", target_bir_lowering=False, debug=False,
                   num_devices=ncores)

    x_sh = nc.dram_tensor("x_sh", [b_sh, in_dim], F32, kind="ExternalInput")
    w_sl_t = nc.dram_tensor("w_sl", [w_sl, in_dim], F32, kind="ExternalInput")
    w_sl2_t = nc.dram_tensor("w_sl2", [w_sl, in_dim], F32,
                             kind="ExternalInput")
    b2_rep = nc.dram_tensor("b2_rep", [P, out_dim], F32, kind="ExternalInput")
    ident_in = nc.dram_tensor("ident", [P, P], BF16, kind="ExternalInput")
    out_sh = nc.dram_tensor("out_sh", [b_sh, out_dim], F32,
                            kind="ExternalOutput")

    wqt_loc = nc.dram_tensor("wqt_loc", [in_dim, NB], BF16)
    wq_ag = nc.dram_tensor("wq_ag", [ncores * in_dim, NB], BF16,
                           addr_space="Shared")
    # warmup-collective scratch: contents never read, only the rendezvous
    # side effect matters (collectives cannot touch IO tensors)
    warm_src = nc.dram_tensor("warm_src", [P, 8], BF16)
    warm_ag = nc.dram_tensor("warm_ag", [ncores * P, 8], BF16,
                             addr_space="Shared")

    with tile.TileContext(nc) as tc:
        from contextlib import ExitStack
        with ExitStack() as ctx:
            qpool = ctx.enter_context(tc.tile_pool(name="qpool", bufs=3))
            spool = ctx.enter_context(tc.tile_pool(name="spool", bufs=3))
            big = ctx.enter_context(tc.tile_pool(name="big", bufs=1))
            wpool = ctx.enter_context(tc.tile_pool(name="wpool", bufs=2))
            opool = ctx.enter_context(tc.tile_pool(name="opool", bufs=3))
            mm_pool = ctx.enter_context(
                tc.tile_pool(name="mmp", bufs=2, space="PSUM"))

            ident = big.tile([P, P], BF16, tag="ident")
            nc.scalar.dma_start(ident[:], ident_in.ap())
            b2_sb = big.tile([P, out_dim], F32, tag="b2_sb")
            nc.scalar.dma_start(b2_sb[:], b2_rep.ap())

            # tiny warmup collective (gpsimd carries only the collectives
            # and post-AG drain multiplies, so nothing local queues behind
            # a collective wait)
            nc.gpsimd.collective_compute(
                "AllGather", ALU.bypass,
                replica_groups=[list(range(ncores))],
                ins=[warm_src.ap().opt()],
                outs=[warm_ag.ap().opt()])

            # ---- quant: half-tiles, phase-split over a pair --------------
            def q_half(src, row, col, tag, queue):
                """Load + start quant chain for one [P, HW] half tile.
                Returns (i8, scale) for the dequant step."""
                xt = qpool.tile([P, HW], F32, tag="xt", name=f"xt_{tag}")
                queue.dma_start(xt[:], src.ap()[row:row + P, col:col + HW])
                return xt

            def q_scales(xt, tag):
                g = HW // SZ
                x3 = xt.rearrange("p (g s) -> p g s", s=SZ)
                m = spool.tile([P, g], F32, tag="q_m", name=f"m_{tag}")
                nc.vector.tensor_reduce(m[:], x3, axis=AX.X, op=ALU.max,
                                        apply_absolute_value=True)
                scale = spool.tile([P, g], F32, tag="q_scale",
                                   name=f"scale_{tag}")
                nc.vector.tensor_scalar(
                    scale[:].bitcast(I32), m[:].bitcast(I32),
                    0x7F800000, None, op0=ALU.bitwise_and)
                nc.vector.tensor_scalar(
                    scale[:].bitcast(I32), scale[:].bitcast(I32),
                    7 << 23, None, op0=ALU.subtract)
                inv = spool.tile([P, g], F32, tag="q_inv", name=f"inv_{tag}")
                nc.vector.tensor_scalar(
                    inv[:].bitcast(I32), scale[:].bitcast(I32),
                    -1, None, op0=ALU.bitwise_xor)
                nc.vector.tensor_scalar(
                    inv[:].bitcast(I32), inv[:].bitcast(I32),
                    (254 << 23) + 1, None, op0=ALU.add)
                return scale, inv

            def q_mult(xt, inv):
                g = HW // SZ
                x3 = xt.rearrange("p (g s) -> p g s", s=SZ)
                nc.vector.tensor_tensor(
                    x3, x3, inv[:, :, None].to_broadcast([P, g, SZ]), ALU.mult)

            def q_int8(xt, tag):
                i8 = qpool.tile([P, HW], I8, tag="i8", bufs=2,
                                name=f"i8_{tag}")
                nc.scalar.activation(i8[:], xt, AF.Copy, bias=0.0, scale=1.0)
                return i8

            def q_deq(i8, scale, tag):
                g = HW // SZ
                q = qpool.tile([P, HW], BF16, tag="q", bufs=2,
                               name=f"q_{tag}")
                nc.vector.tensor_tensor(
                    q[:].rearrange("p (g s) -> p g s", s=SZ),
                    i8.rearrange("p (g s) -> p g s", s=SZ),
                    scale[:, :, None].to_broadcast([P, g, SZ]), ALU.mult)
                return q

            def quant_pair(src, rowcols, tag, queue):
                """Quantize two [P, HW] half tiles, phase-interleaved.
                Returns the bf16 q half-tiles."""
                xts = [q_half(src, r, c, f"{tag}{i}", queue)
                       for i, (r, c) in enumerate(rowcols)]
                si = [q_scales(xt[:], f"{tag}{i}")
                      for i, xt in enumerate(xts)]
                for i, xt in enumerate(xts):
                    q_mult(xt[:], si[i][1][:])
                i8s = [q_int8(xts[i][:], f"{tag}{i}") for i in range(2)]
                return [q_deq(i8s[i][:], si[i][0][:], f"{tag}{i}")
                        for i in range(2)]

            # transpose one bf16 q half-tile (16 k-chunks) into `sink`:
            # sink[:, k0+kq*4:(kq+1)*4, cc:cc+P] gets the transposed chunks
            def half_transpose(q, k0, sink, cc, tag, copy_eng):
                for kq in range(kh // 4):
                    tp = mm_pool.tile([P, 4, P], BF16, tag="tp", bufs=4,
                                      name=f"tp_{tag}_{kq}")
                    for t in range(4):
                        k = kq * 4 + t
                        nc.tensor.transpose(tp[:, t, :],
                                            q[:, k * P:(k + 1) * P], ident[:])
                    copy_eng.copy(
                        sink[:, k0 + kq * 4:k0 + (kq + 1) * 4, cc:cc + P],
                        tp[:])

            # ---- W contribution block: quant + transpose + store + AG ----
            # ALL quant-input loads ride the Sync queue (and nothing else
            # does): the tile scheduler orders a queue by modeled readiness,
            # so a queue mixing pre-AG loads with post-AG loads can end up
            # with every pre-AG load stuck behind the collective wait.
            for i in range(n_wt):
                qs = quant_pair(w_sl_t, [(i * P, 0), (i * P, HW)], f"wc{i}",
                                nc.sync)
                wtt = wpool.tile([P, kc, P], BF16, tag="w", name=f"wtt_{i}")
                for h, q in enumerate(qs):
                    half_transpose(q[:], h * kh, wtt, 0, f"wc{i}{h}",
                                   nc.scalar)
                nc.scalar.dma_start(
                    wqt_loc.ap()[:, i * P:(i + 1) * P].rearrange(
                        "(k p) o -> p k o", p=P),
                    wtt[:, :, 0:P])

            # main AllGather
            nc.gpsimd.collective_compute(
                "AllGather", ALU.bypass,
                replica_groups=[list(range(ncores))],
                ins=[wqt_loc.ap().opt()],
                outs=[wq_ag.ap().opt()])

            # ---- local W block (block 7, identical on every core) --------
            slabL = wpool.tile([P, kc, NB], BF16, tag="w", name="slabL")
            for i in range(n_wt):
                qs = quant_pair(w_sl2_t, [(i * P, 0), (i * P, HW)], f"wl{i}",
                                nc.sync)
                for h, q in enumerate(qs):
                    half_transpose(q[:], h * kh, slabL, i * P, f"wl{i}{h}",
                                   nc.scalar)

            # ---- x quant + transpose into resident xqT -------------------
            xqT = big.tile([P, kc, b_sh], BF16, tag="xqT")
            for bb in range(n_xt):
                qs = quant_pair(x_sh, [(bb * P, 0), (bb * P, HW)], f"x{bb}",
                                nc.sync)
                for h, q in enumerate(qs):
                    half_transpose(q[:], h * kh, xqT, bb * P, f"x{bb}{h}",
                                   nc.scalar)

            # ---- matmul sweep --------------------------------------------
            def drain(ps, bb, jcol, mult_eng):
                s = opool.tile([P, NB], F32, tag="ds", name=f"s_{jcol}_{bb}")
                nc.scalar.copy(s[:], ps[:])   # releases the PSUM bank
                nc.vector.tensor_tensor(
                    s[:], s[:], b2_sb[:, jcol * NB:(jcol + 1) * NB], ALU.add)
                g = NB // SZ
                s3 = s[:].rearrange("p (g s) -> p g s", s=SZ)
                m = opool.tile([P, g], F32, tag="o_m", name=f"om_{jcol}_{bb}")
                nc.vector.tensor_reduce(m[:], s3, axis=AX.X, op=ALU.max,
                                        apply_absolute_value=True)
                scale = opool.tile([P, g], F32, tag="o_scale",
                                   name=f"osc_{jcol}_{bb}")
                nc.vector.tensor_scalar(
                    scale[:].bitcast(I32), m[:].bitcast(I32),
                    0x7F800000, None, op0=ALU.bitwise_and)
                nc.vector.tensor_scalar(
                    scale[:].bitcast(I32), scale[:].bitcast(I32),
                    7 << 23, None, op0=ALU.subtract)
                inv = opool.tile([P, g], F32, tag="o_inv",
                                 name=f"oin_{jcol}_{bb}")
                nc.vector.tensor_scalar(
                    inv[:].bitcast(I32), scale[:].bitcast(I32),
                    -1, None, op0=ALU.bitwise_xor)
                nc.vector.tensor_scalar(
                    inv[:].bitcast(I32), inv[:].bitcast(I32),
                    (254 << 23) + 1, None, op0=ALU.add)
                mult_eng.tensor_tensor(
                    s3, s3, inv[:, :, None].to_broadcast([P, g, SZ]),
                    ALU.mult)
                oi8 = opool.tile([P, NB], I8, tag="oi8",
                                 name=f"oi8_{jcol}_{bb}")
                nc.scalar.activation(oi8[:], s[:], AF.Copy, bias=0.0,
                                     scale=1.0)
                oq = opool.tile([P, NB], F32, tag="oq",
                                name=f"oq_{jcol}_{bb}")
                nc.vector.tensor_tensor(
                    oq[:].rearrange("p (g s) -> p g s", s=SZ),
                    oi8[:].rearrange("p (g s) -> p g s", s=SZ),
                    scale[:, :, None].to_broadcast([P, g, SZ]), ALU.mult)
                nc.scalar.dma_start(
                    out_sh.ap()[bb * P:(bb + 1) * P,
                                jcol * NB:(jcol + 1) * NB],
                    oq[:])

            for t in range(ncores):
                if t == 0:
                    slab = slabL
                    jcol = LOCAL_BLK
                else:
                    jcol = t - 1
                    slab = wpool.tile([P, kc, NB], BF16, tag="w",
                                      name=f"slab_{t}")
                    # slab loads ride gpsimd: everything there is post-AG
                    nc.gpsimd.dma_start(
                        slab[:],
                        wq_ag.ap()[jcol * in_dim:(jcol + 1) * in_dim, :]
                        .rearrange("(k p) o -> p k o", p=P))
                for bb in range(n_xt):
                    ps = mm_pool.tile([P, NB], F32, tag="ps", bufs=3,
                                      name=f"ps_{t}_{bb}")
                    for k in range(kc):
                        nc.tensor.matmul(
                            ps[:],
                            lhsT=xqT[:, k, bb * P:(bb + 1) * P],
                            rhs=slab[:, k, :],
                            start=(k == 0), stop=(k == kc - 1),
                            skip_group_check=True)
                    # t=0 drains run pre-AG: keep them off the gpsimd
                    # queue (which is waiting on the AllGather)
                    drain(ps, bb, jcol,
                          nc.vector if t == 0 else nc.gpsimd)

    nc.compile()
    return nc


_NC_CACHE = {}


def _get_nc(key=(B_FULL // NCORES, IN_FULL, OUT_FULL, NCORES)):
    if key not in _NC_CACHE:
        _NC_CACHE[key] = build_nc(*key)
    return _NC_CACHE[key]


def make_in_maps(x, W, b, ncores=NCORES):
    import ml_dtypes
    b_sh = x.shape[0] // ncores
    w_sl = W.shape[0] // ncores
    out_dim = W.shape[0]
    b2 = (2.0 * np.asarray(b, np.float32)).astype(np.float32)
    b2_rep = np.ascontiguousarray(
        np.broadcast_to(b2.reshape(1, out_dim), (P, out_dim)))
    ident = np.eye(P, dtype=ml_dtypes.bfloat16)
    w_local = np.ascontiguousarray(
        W[LOCAL_BLK * w_sl:(LOCAL_BLK + 1) * w_sl])
    return [
        {
            "x_sh": np.ascontiguousarray(x[c * b_sh:(c + 1) * b_sh]),
            "w_sl": np.ascontiguousarray(W[c * w_sl:(c + 1) * w_sl]),
            "w_sl2": w_local,
            "b2_rep": b2_rep,
            "ident": ident,
        }
        for c in range(ncores)
    ]


def kernel(x, W, b):
    from concourse.bass_utils import run_bass_kernel_spmd

    x = np.asarray(x, np.float32)
    W = np.asarray(W, np.float32)
    b = np.asarray(b, np.float32)
    nc = _get_nc()
    in_maps = make_in_maps(x, W, b)
    res = run_bass_kernel_spmd(nc, in_maps, core_ids=list(range(NCORES)))
    return np.concatenate([res.results[c]["out_sh"] for c in range(NCORES)],
                          axis=0)
